# revision 14
# baseline (speedup 1.0000x reference)
"""Multi-head self-attention Trainium2 kernel (8-core tensor parallel).

Problem: x[1, 4096, 1024], 16 heads, d_head=64, softmax(q k^T / 8) v, out proj.
Sharding: 2 heads per core; w_out row-sharded; host sums the 8 output partials.

Per-core device kernel (all matmul operands bf16, fp32 PSUM accumulation):
  phase 1: load x^T (bf16) resident in SBUF
  phase 2: Q^T, K^T = w^T-slices @ x^T (+bias, q pre-scaled by 1/8 on host);
           V in natural [n, d] layout, stored ones-augmented per head
  phase 3: per (q-chunk, m-block, head): S^T = K^T_blk^T-contracted matmul,
           P^T = exp(S^T) on ACT (no max subtraction: scores are ~N(0,1)),
           C^T[d+1, q] += [V|1]^T @ P^T  (row d = softmax denominators)
           then ctx^T = C^T[:d] * (1/C^T[d]) broadcast
  phase 4: out[q, :] = ctx^T-contracted matmul with w_out slice
"""

import numpy as np
import ml_dtypes

import concourse.bacc as bacc
import concourse.mybir as mybir
import concourse.tile as tile
from concourse.bass_utils import run_bass_kernel_spmd

BF16 = mybir.dt.bfloat16
F32 = mybir.dt.float32
NP_BF16 = ml_dtypes.bfloat16

N_CORES = 8
SEQ = 4096
D_MODEL = 1024
D_HEAD = 64
SCALE = 8.0  # sqrt(1024 / 16)


def build_core_kernel(n=SEQ, d_model=D_MODEL, num_devices=N_CORES, qc=1024,
                      debug_taps=False):
    """One core's kernel: 2 heads of attention + its slice of the out proj."""
    P = 128
    ko = d_model // P          # contraction tiles over d_model
    qc = min(qc, n)            # attention q-chunk
    nj = n // qc
    mb = n // P                # m (key position) blocks
    Exp = mybir.ActivationFunctionType.Exp

    nc = bacc.Bacc("TRN2", target_bir_lowering=False, debug=False,
                   num_devices=num_devices)
    xT = nc.dram_tensor("xT", [d_model, n], BF16, kind="ExternalInput").ap()
    wT = nc.dram_tensor("wT", [d_model, 384], BF16, kind="ExternalInput").ap()
    bqk = nc.dram_tensor("bqk", [256], F32, kind="ExternalInput").ap()
    wo = nc.dram_tensor("wo", [P, d_model], BF16, kind="ExternalInput").ap()
    out = nc.dram_tensor("out", [n, d_model], F32, kind="ExternalOutput").ap()
    taps = {}
    if debug_taps:
        for tname, shape in (("dQT", [P, n]), ("dKT", [P, n]),
                             ("dvaug", [P, (n // P) * 256]),
                             ("dctxT", [P, n])):
            taps[tname] = nc.dram_tensor(
                tname, shape, BF16, kind="ExternalOutput").ap()
        taps["dC"] = nc.dram_tensor(
            "dC", [P, 2 * min(qc, n)], F32, kind="ExternalOutput").ap()
        taps["drec"] = nc.dram_tensor(
            "drec", [64, 2 * min(qc, n)], F32, kind="ExternalOutput").ap()

    with tile.TileContext(nc) as tc:
        with tc.tile_pool(name="persist", bufs=1) as pp:
            wT_s = pp.tile([P, ko, 384], BF16)
            nc.sync.dma_start(wT_s[:], wT.rearrange("(ko p) f -> p ko f", p=P))
            wo_s = pp.tile([P, d_model], BF16)
            nc.sync.dma_start(wo_s[:], wo)
            bqk_s = pp.tile([P, 2], F32)
            nc.sync.dma_start(bqk_s[:], bqk.rearrange("(g p) -> p g", p=P))
            xT_s = pp.tile([P, ko, n], BF16)
            for k in range(ko):
                nc.sync.dma_start(xT_s[:, k, :], xT[k * P:(k + 1) * P, :])

            QT = pp.tile([P, n], BF16)
            KT = pp.tile([P, n], BF16)
            # per m-block: [V_h0(64) | ones(64) | V_h1(64) | ones(64)];
            # the ones columns make the PV matmul emit softmax denominators
            # on output partitions 64..127.
            vaug = pp.tile([P, mb * 256], BF16)
            ctxT = pp.tile([P, n], BF16)
            nc.vector.memset(vaug[:], 1.0)

            # ---- phase 2: projections ----
            with tc.tile_pool(name="proj_ps", bufs=4, space="PSUM") as pjp:
                for t, dst in ((0, QT), (1, KT)):
                    for j in range(n // 512):
                        ps = pjp.tile([P, 512], F32, tag="qk")
                        for k in range(ko):
                            nc.tensor.matmul(
                                ps[:],
                                wT_s[:, k, t * P:(t + 1) * P],
                                xT_s[:, k, j * 512:(j + 1) * 512],
                                start=(k == 0), stop=(k == ko - 1))
                        nc.vector.tensor_scalar_add(
                            dst[:, j * 512:(j + 1) * 512], ps[:],
                            bqk_s[:, t:t + 1])
                for b in range(mb):
                    ps = pjp.tile([P, P], F32, tag="v")
                    for k in range(ko):
                        nc.tensor.matmul(
                            ps[:],
                            xT_s[:, k, b * P:(b + 1) * P],
                            wT_s[:, k, 256:384],
                            start=(k == 0), stop=(k == ko - 1))
                    nc.vector.tensor_copy(
                        vaug[:, b * 256:b * 256 + 64], ps[:, 0:64])
                    nc.vector.tensor_copy(
                        vaug[:, b * 256 + 128:b * 256 + 192], ps[:, 64:128])

            # ---- phase 3: attention ----
            with tc.tile_pool(name="st_ps", bufs=2, space="PSUM") as stp, \
                 tc.tile_pool(name="ctx_ps", bufs=1, space="PSUM") as cxp, \
                 tc.tile_pool(name="pt_sb", bufs=3) as ptp, \
                 tc.tile_pool(name="rec_sb", bufs=2) as rcp:
                for j in range(nj):
                    ctx = [cxp.tile([P, qc], F32, tag=f"ctx{hh}",
                                    name=f"ctx{hh}_{j}")
                           for hh in (0, 1)]
                    for b in range(mb):
                        for hh in (0, 1):
                            st = stp.tile([P, qc], F32, tag="st")
                            for h2 in range(qc // 512):
                                nc.tensor.matmul(
                                    st[:, h2 * 512:(h2 + 1) * 512],
                                    KT[hh * 64:(hh + 1) * 64,
                                       b * P:(b + 1) * P],
                                    QT[hh * 64:(hh + 1) * 64,
                                       j * qc + h2 * 512:
                                       j * qc + (h2 + 1) * 512],
                                    start=True, stop=True)
                            pt = ptp.tile([P, qc], BF16, tag="pt")
                            nc.scalar.activation(pt[:], st[:], Exp)
                            va = vaug[:, b * 256 + hh * 128:
                                      b * 256 + (hh + 1) * 128]
                            for h2 in range(qc // 512):
                                nc.tensor.matmul(
                                    ctx[hh][:, h2 * 512:(h2 + 1) * 512],
                                    va,
                                    pt[:, h2 * 512:(h2 + 1) * 512],
                                    start=(b == 0), stop=(b == mb - 1))
                    for hh in (0, 1):
                        rec = rcp.tile([64, qc], F32, tag="rec")
                        nc.vector.reciprocal(rec[:], ctx[hh][64:128, :])
                        if debug_taps and j == nj - 1:
                            csb = rcp.tile([P, qc], F32, tag="csb",
                                           name=f"csb{hh}")
                            nc.vector.tensor_copy(csb[:], ctx[hh][:])
                            nc.sync.dma_start(
                                taps["dC"][:, hh * qc:(hh + 1) * qc], csb[:])
                            nc.sync.dma_start(
                                taps["drec"][:, hh * qc:(hh + 1) * qc],
                                rec[:])
                        nc.vector.tensor_mul(
                            ctxT[hh * 64:(hh + 1) * 64, j * qc:(j + 1) * qc],
                            ctx[hh][0:64, :],
                            rec[:])

            if debug_taps:
                nc.sync.dma_start(taps["dQT"], QT[:])
                nc.sync.dma_start(taps["dKT"], KT[:])
                nc.sync.dma_start(taps["dvaug"], vaug[:])
                nc.sync.dma_start(taps["dctxT"], ctxT[:])

            # ---- phase 4: output projection ----
            with tc.tile_pool(name="out_ps", bufs=4, space="PSUM") as oup, \
                 tc.tile_pool(name="out_sb", bufs=3) as osp:
                oc = min(512, d_model)
                for qb in range(mb):
                    ost = osp.tile([P, d_model], F32, tag="ost")
                    for ch in range(d_model // oc):
                        po = oup.tile([P, oc], F32, tag="po")
                        nc.tensor.matmul(
                            po[:],
                            ctxT[:, qb * P:(qb + 1) * P],
                            wo_s[:, ch * oc:(ch + 1) * oc],
                            start=True, stop=True)
                        nc.vector.tensor_copy(
                            ost[:, ch * oc:(ch + 1) * oc], po[:])
                    nc.sync.dma_start(out[qb * P:(qb + 1) * P, :], ost[:])

    nc.compile()
    return nc


def make_in_maps(x, w_qkv, b_qkv, w_out, n_cores=N_CORES):
    """Shard full inputs into per-core input maps (head-parallel)."""
    d = x.shape[-1]
    xTb = np.ascontiguousarray(x.reshape(-1, d).T).astype(NP_BF16)
    in_maps = []
    for c in range(n_cores):
        r = slice(c * 128, (c + 1) * 128)
        wq = w_qkv[0 * d:1 * d][r] / SCALE
        wk = w_qkv[1 * d:2 * d][r]
        wv = w_qkv[2 * d:3 * d][r]
        wTb = np.ascontiguousarray(
            np.concatenate([wq, wk, wv], axis=0).T).astype(NP_BF16)
        bqkb = np.concatenate(
            [b_qkv[0 * d:1 * d][r] / SCALE,
             b_qkv[1 * d:2 * d][r]]).astype(np.float32)
        wob = np.ascontiguousarray(w_out[:, r].T).astype(NP_BF16)
        in_maps.append({"xT": xTb, "wT": wTb, "bqk": bqkb, "wo": wob})
    return in_maps


def gather_output(results, w_qkv, b_qkv, w_out):
    """Sum per-core partials; add out bias and the folded v-bias term."""
    d = w_out.shape[0]
    acc = np.zeros(results[0]["out"].shape, np.float64)
    for r in results:
        acc += r["out"].astype(np.float64)
    bv = b_qkv[2 * d:3 * d].astype(np.float64)
    const = w_out.astype(np.float64) @ bv
    return acc + const


_NC_CACHE = {}


def kernel(**inputs):
    x = np.asarray(inputs["x"], dtype=np.float32)
    w_qkv = np.asarray(inputs["w_qkv"], dtype=np.float32)
    b_qkv = np.asarray(inputs["b_qkv"], dtype=np.float32)
    w_out = np.asarray(inputs["w_out"], dtype=np.float32)
    b_out = np.asarray(inputs["b_out"], dtype=np.float32)
    b, n, d = x.shape

    key = (n, d)
    if key not in _NC_CACHE:
        _NC_CACHE[key] = build_core_kernel(n=n, d_model=d)
    nc = _NC_CACHE[key]

    in_maps = make_in_maps(x, w_qkv, b_qkv, w_out)
    res = run_bass_kernel_spmd(nc, in_maps, core_ids=list(range(N_CORES)))

    acc = gather_output(res.results, w_qkv, b_qkv, w_out)
    out = (acc + b_out.astype(np.float64)).astype(np.float32)
    return out.reshape(b, n, d)
